# revision 7
# baseline (speedup 1.0000x reference)
"""Contrastive learning loss (supervised NT-Xent style) on 8 Trainium2 NeuronCores.

Full inputs in, full output out.  Sharding: embeddings are row-sharded over
batch across the 8 cores (1024 query rows each); every core receives the full
embeddings tensor (host-side replication = free all-gather) and computes the
row-parallel BxB softmax statistics for its rows only.

Per-row math (T = temperature):
    en'   = en / max(||en||,1e-12) * (1/sqrt(T))      so  sim = en'_q . en'_j
    lse_q = ln(sum_j exp(sim_qj))                     (no max needed: |sim|<=1/T)
    s_q   = sum_{j: lab_j==lab_q, j!=q} sim_qj = en'_q . csum[lab_q] - 1/T
    c_q   = hist[lab_q] - 1
    loss  = mean_q  (lse_q - s_q/max(c_q,1)) * min(c_q,1)

csum[c] (class-summed normalized embeddings, [1024, 257] with a count column)
is computed per-core over its local rows via a one-hot matmul and AllReduce'd
across the 8 cores, then fetched per query row by indirect-DMA gather.
"""

import math
import os
from contextlib import ExitStack

import numpy as np

import concourse.bass as bass
import concourse.bacc as bacc
import concourse.tile as tile
from concourse import mybir
from concourse.bass import ds, ts
from concourse.bass_utils import run_bass_kernel_spmd
from concourse.masks import make_identity

N_CORES = 8
B = 8192
D = 256
NCLS = 1024
BQ = B // N_CORES          # query rows per core
NT_Q = BQ // 128           # 8 query tiles per core
NT = B // 128              # 64 row tiles total
CH = 8                     # prep chunks
TPC = NT // CH             # tiles per prep chunk

TEMP = 0.07
SCALE = 1.0 / math.sqrt(TEMP)
NEG_INV_T = -1.0 / TEMP

F32 = mybir.dt.float32
BF16 = mybir.dt.bfloat16
I32 = mybir.dt.int32
ALU = mybir.AluOpType
ACTF = mybir.ActivationFunctionType
AX = mybir.AxisListType

_CACHE = {}


def _build_nc():
    nc = bacc.Bacc(
        "TRN2", target_bir_lowering=False, debug=False, num_devices=N_CORES
    )

    emb = nc.dram_tensor("emb_full", [B, D], F32, kind="ExternalInput")
    qemb = nc.dram_tensor("q_emb", [BQ, D], F32, kind="ExternalInput")
    labf = nc.dram_tensor("lab_q_f", [128, NT_Q], F32, kind="ExternalInput")
    labi = nc.dram_tensor("lab_q_i", [128, NT_Q], I32, kind="ExternalInput")
    lossout = nc.dram_tensor("loss_out", [128, NT_Q], F32, kind="ExternalOutput")

    with tile.TileContext(nc) as tc, ExitStack() as ctx:
        const = ctx.enter_context(tc.tile_pool(name="const", bufs=1))
        big = ctx.enter_context(tc.tile_pool(name="big", bufs=1))
        work = ctx.enter_context(tc.tile_pool(name="work", bufs=2))
        small = ctx.enter_context(tc.tile_pool(name="small", bufs=4))
        dram = ctx.enter_context(tc.tile_pool(name="dram", bufs=1, space="DRAM"))

        # ---- constants ----
        iota_i = const.tile([128, NCLS], I32)
        nc.gpsimd.iota(iota_i[:], pattern=[[1, NCLS]], base=0, channel_multiplier=0)
        iota_f = const.tile([128, NCLS], F32)
        nc.vector.tensor_copy(out=iota_f[:], in_=iota_i[:])
        ident = const.tile([128, 128], BF16)
        make_identity(nc, ident[:])

        # ---- persistent buffers ----
        en_bf = big.tile([128, NT, D], BF16)        # normalized*SCALE, natural layout
        enT0 = big.tile([128, B], BF16)             # en'[:, 0:128].T   (d-low on partitions)
        enT1 = big.tile([128, B], BF16)             # en'[:, 128:256].T
        q_aug = big.tile([128, NT_Q, D + 1], BF16)  # local rows, + ones column
        qT0 = big.tile([128, BQ], BF16)
        qT1 = big.tile([128, BQ], BF16)
        oh = big.tile([128, NT_Q, NCLS], BF16)      # one-hot of local labels
        csum_sb = big.tile([128, NT_Q, D + 1], F32)
        labf_sb = big.tile([128, NT_Q], F32)
        labi_sb = big.tile([128, NT_Q], I32)
        loss_sb = big.tile([128, NT_Q], F32)
        q_nat = big.tile([128, NT_Q, D], F32)

        cc_in = dram.tile([NCLS, D + 1], F32)
        cc_out = dram.tile([NCLS, D + 1], F32)

        nc.sync.dma_start(out=labf_sb[:], in_=labf[:])
        nc.sync.dma_start(out=labi_sb[:], in_=labi[:])
        nc.sync.dma_start(
            out=q_nat[:], in_=qemb[:].rearrange("(t p) d -> p t d", p=128)
        )

        # ---- local (query-side) normalization ----
        sq_q = work.tile([128, NT_Q, D], F32, tag="sq")
        nc.scalar.square(out=sq_q[:], in_=q_nat[:])
        ssq_q = small.tile([128, NT_Q], F32, tag="ssq")
        nc.vector.reduce_sum(ssq_q[:], sq_q[:], axis=AX.X)
        nc.vector.tensor_scalar_max(out=ssq_q[:], in0=ssq_q[:], scalar1=1e-24)
        nc.scalar.activation(out=ssq_q[:], in_=ssq_q[:], func=ACTF.Ln)
        inv_q = small.tile([128, NT_Q], F32, tag="invc")
        nc.scalar.activation(out=inv_q[:], in_=ssq_q[:], func=ACTF.Exp, scale=-0.5)
        for t in range(NT_Q):
            nc.vector.tensor_scalar(
                out=q_aug[:, t, 0:D],
                in0=q_nat[:, t, :],
                scalar1=inv_q[:, t : t + 1],
                scalar2=SCALE,
                op0=ALU.mult,
                op1=ALU.mult,
            )
        nc.vector.memset(q_aug[:, :, D : D + 1], 1.0)
        for t in range(NT_Q):
            nc.vector.tensor_scalar(
                out=oh[:, t, :],
                in0=iota_f[:],
                scalar1=labf_sb[:, t : t + 1],
                scalar2=None,
                op0=ALU.is_equal,
            )

        with (
            tc.tile_pool(name="tpsum", bufs=2, space="PSUM") as tp,
            tc.tile_pool(name="cpsum", bufs=2, space="PSUM") as cp,
        ):
            # ---- query-side transposes -> qT0/qT1 ----
            for g in range(NT_Q // 4):
                for half, qT in ((0, qT0), (1, qT1)):
                    pt = tp.tile([128, 512], BF16, tag="tp")
                    for k in range(4):
                        t = g * 4 + k
                        nc.tensor.transpose(
                            pt[:, ts(k, 128)],
                            q_aug[:, t, half * 128 : half * 128 + 128],
                            ident[:],
                        )
                    nc.vector.tensor_copy(out=qT[:, ts(g, 512)], in_=pt[:])

            # ---- local class sums (csumT [1024, 257]) + AllReduce ----
            for mc in range(NCLS // 128):
                pc = cp.tile([128, D + 1], F32, tag="cp")
                for jc in range(NT_Q):
                    nc.tensor.matmul(
                        pc[:],
                        lhsT=oh[:, jc, ts(mc, 128)],
                        rhs=q_aug[:, jc, :],
                        start=(jc == 0),
                        stop=(jc == NT_Q - 1),
                    )
                nc.vector.tensor_copy(out=csum_sb[:, mc, :], in_=pc[:])
            nc.sync.dma_start(
                out=cc_in[:].rearrange("(m p) n -> p m n", p=128), in_=csum_sb[:]
            )
            if os.environ.get("BASSK_NO_CC"):
                nc.sync.dma_start(out=cc_out[:], in_=cc_in[:])
            else:
                nc.gpsimd.collective_compute(
                    "AllReduce",
                    ALU.add,
                    replica_groups=[list(range(N_CORES))],
                    ins=[cc_in[:]],
                    outs=[cc_out[:]],
                )

            # ---- full-set prep: load, normalize, transpose into enT0/enT1 ----
            emb_r = emb[:].rearrange("(c t p) d -> c p t d", t=TPC, p=128)
            for ch in range(CH):
                fx = work.tile([128, TPC, D], F32, tag="fx")
                nc.sync.dma_start(out=fx[:], in_=emb_r[ch])
                sqc = work.tile([128, TPC, D], F32, tag="sq")
                nc.scalar.square(out=sqc[:], in_=fx[:])
                ssq = small.tile([128, TPC], F32, tag="ssq")
                nc.vector.reduce_sum(ssq[:], sqc[:], axis=AX.X)
                nc.vector.tensor_scalar_max(out=ssq[:], in0=ssq[:], scalar1=1e-24)
                nc.scalar.activation(out=ssq[:], in_=ssq[:], func=ACTF.Ln)
                invc = small.tile([128, TPC], F32, tag="invc")
                nc.scalar.activation(out=invc[:], in_=ssq[:], func=ACTF.Exp, scale=-0.5)
                for tt in range(TPC):
                    tg = ch * TPC + tt
                    nc.vector.tensor_scalar(
                        out=en_bf[:, tg, :],
                        in0=fx[:, tt, :],
                        scalar1=invc[:, tt : tt + 1],
                        scalar2=SCALE,
                        op0=ALU.mult,
                        op1=ALU.mult,
                    )
                for g in range(TPC // 4):
                    for half, eT in ((0, enT0), (1, enT1)):
                        pt = tp.tile([128, 512], BF16, tag="tp")
                        for k in range(4):
                            tg = ch * TPC + g * 4 + k
                            nc.tensor.transpose(
                                pt[:, ts(k, 128)],
                                en_bf[:, tg, half * 128 : half * 128 + 128],
                                ident[:],
                            )
                        nc.vector.tensor_copy(
                            out=eT[:, ds(ch * TPC * 128 + g * 512, 512)], in_=pt[:]
                        )

        # ---- main loop: row-parallel softmax stats ----
        with (
            tc.tile_pool(name="mpsum", bufs=2, space="PSUM") as mpp,
            tc.tile_pool(name="fin", bufs=4) as fin,
        ):
            for t in range(NT_Q):
                esum = fin.tile([128, 4], F32, tag="esum")
                for h in range(4):
                    pm = mpp.tile([128, 2048], F32, tag="mp")
                    for c in range(4):
                        n0 = (h * 4 + c) * 512
                        nc.tensor.matmul(
                            pm[:, ts(c, 512)],
                            lhsT=qT0[:, ts(t, 128)],
                            rhs=enT0[:, ds(n0, 512)],
                            start=True,
                            stop=False,
                        )
                        nc.tensor.matmul(
                            pm[:, ts(c, 512)],
                            lhsT=qT1[:, ts(t, 128)],
                            rhs=enT1[:, ds(n0, 512)],
                            start=False,
                            stop=True,
                        )
                    nc.scalar.activation(
                        out=pm[:],
                        in_=pm[:],
                        func=ACTF.Exp,
                        accum_out=esum[:, h : h + 1],
                    )
                se = fin.tile([128, 1], F32, tag="se")
                nc.vector.reduce_sum(se[:], esum[:], axis=AX.X)
                lse = fin.tile([128, 1], F32, tag="lse")
                nc.scalar.activation(out=lse[:], in_=se[:], func=ACTF.Ln)

                gath = fin.tile([128, D + 1], F32, tag="gath")
                nc.gpsimd.indirect_dma_start(
                    out=gath[:],
                    out_offset=None,
                    in_=cc_out[:],
                    in_offset=bass.IndirectOffsetOnAxis(
                        ap=labi_sb[:, t : t + 1], axis=0
                    ),
                )
                scr = fin.tile([128, D], F32, tag="scr")
                nc.vector.tensor_mul(out=scr[:], in0=q_aug[:, t, 0:D], in1=gath[:, 0:D])
                s_acc = fin.tile([128, 1], F32, tag="sacc")
                nc.vector.reduce_sum(s_acc[:], scr[:], axis=AX.X)
                cm1 = fin.tile([128, 1], F32, tag="cm1")
                nc.vector.tensor_scalar_add(out=cm1[:], in0=gath[:, D : D + 1], scalar1=-1.0)
                icm = fin.tile([128, 1], F32, tag="icm")
                nc.vector.tensor_scalar_max(out=icm[:], in0=cm1[:], scalar1=1.0)
                nc.vector.reciprocal(out=icm[:], in_=icm[:])
                ind = fin.tile([128, 1], F32, tag="ind")
                nc.vector.tensor_scalar_min(out=ind[:], in0=cm1[:], scalar1=1.0)
                pos = fin.tile([128, 1], F32, tag="pos")
                # pos = (s_acc - 1/T) * (1/max(c-1,1)); the -1/T removes the diagonal term
                nc.vector.scalar_tensor_tensor(
                    out=pos[:],
                    in0=s_acc[:],
                    scalar=NEG_INV_T,
                    in1=icm[:],
                    op0=ALU.add,
                    op1=ALU.mult,
                )
                lm = fin.tile([128, 1], F32, tag="lm")
                nc.vector.tensor_sub(out=lm[:], in0=lse[:], in1=pos[:])
                nc.vector.tensor_mul(
                    out=loss_sb[:, t : t + 1], in0=lm[:], in1=ind[:]
                )
            nc.sync.dma_start(out=lossout[:], in_=loss_sb[:])

    nc.finalize()
    return nc


def _get_nc():
    if "nc" not in _CACHE:
        _CACHE["nc"] = _build_nc()
    return _CACHE["nc"]


def _execute(embeddings, labels, trace=False):
    emb = np.ascontiguousarray(np.asarray(embeddings, dtype=np.float32))
    lab = np.asarray(labels)
    labf = lab.astype(np.float32)
    labi = lab.astype(np.int32)
    in_maps = []
    for i in range(N_CORES):
        sl = slice(i * BQ, (i + 1) * BQ)
        in_maps.append(
            {
                "emb_full": emb,
                "q_emb": np.ascontiguousarray(emb[sl]),
                "lab_q_f": np.ascontiguousarray(labf[sl].reshape(NT_Q, 128).T),
                "lab_q_i": np.ascontiguousarray(labi[sl].reshape(NT_Q, 128).T),
            }
        )
    nc = _get_nc()
    res = run_bass_kernel_spmd(
        nc, in_maps, core_ids=list(range(N_CORES)), trace=trace
    )
    rows = np.concatenate(
        [r["loss_out"].T.reshape(-1) for r in res.results]
    )  # row order: core, tile, partition
    loss = np.array(rows.mean(), dtype=np.float32)
    return loss, res


def kernel(embeddings, labels):
    loss, _ = _execute(embeddings, labels, trace=False)
    return loss
